# revision 1
# baseline (speedup 1.0000x reference)
"""Trainium2 Bass kernel for nn_Closing (learnable morphological closing).

reference:
  dilated[b,o,i,j] = sum_c max_{di,dj}( x_pad[b,c,i+di,j+dj] + w_d[o,c,di,dj] )
  out[b,o,i,j]     = min_{di,dj}( dil_pad[b,o,i+di,j+dj] - w_e[o,di,dj] )
  (edge/replicate padding, K=5, PAD=2)

Sharding: 8 cores = 4 batches x 2 halves of the 16 output channels.
Per-core layout: partition p = (o_local, rb) with 8 local output channels and
16 row-blocks of 16 rows; free dim = 16 rows x 256 cols = 4096 elems.

Per tap: "+w" add on ScalarE (Identity + per-partition bias) or VectorE
tensor_scalar (4x for fp16), then a VectorE tensor_tensor max/min chain (2x
for fp16). Intermediate y is re-haloed with SBUF->SBUF DMAs (no DRAM trip).
"""

import numpy as np

import concourse.bass as bass
import concourse.tile as tile
from concourse import bacc, mybir
from concourse.bass_utils import run_bass_kernel_spmd

B, CIN, COUT, H, W, K = 4, 3, 16, 256, 256, 5
PAD = K // 2
Hp = H + 2 * PAD  # 260
NCORES = 8
O_PER = 8    # output channels per core
RB = 16      # row blocks  (O_PER * RB = 128 partitions)
RPB = 16     # rows per block
FD = RPB * W  # 4096 free elems per instruction

TAPS = [(di, dj) for di in range(K) for dj in range(K)]

# engine-time model (ns) used only to pick the ACT/DVE split of the adds
_ACT_NS = (FD + 352) / 1.2
_TT_NS = (58 + FD / 2) / 0.96
_TS_NS = (58 + FD / 4) / 0.96


def _pick_dve_taps():
    """Choose which of the 4x25 adds run as VectorE tensor_scalar (rest on
    ScalarE). Only even-dj taps are eligible (4B alignment keeps the 4x
    mode). Returns set of (group, tap_idx); group 0..2 = dilation c, 3 = ero.
    Tap 0 of each group initializes the accumulator (no tensor_tensor)."""
    n_groups = 4
    total_adds = 25 * n_groups
    fixed_dve = 98 * _TT_NS  # 24 maxes/group + 2 merges
    # balance: fixed + n*TS == (total - n)*ACT
    n = int((total_adds * _ACT_NS - fixed_dve) / (_ACT_NS + _TS_NS))
    n = max(0, min(n, 15 * n_groups))
    per_group = [n // n_groups + (1 if g < n % n_groups else 0)
                 for g in range(n_groups)]
    chosen = set()
    for g in range(n_groups):
        even = [t for t, (di, dj) in enumerate(TAPS) if dj % 2 == 0]
        # spread across the tap sequence
        k = per_group[g]
        if k:
            idxs = np.linspace(0, len(even) - 1, k).round().astype(int)
            for i in idxs:
                chosen.add((g, even[i]))
    return chosen


def build(reps: int = 1, dt=mybir.dt.float16):
    DT = dt
    nc = bacc.Bacc("TRN2", target_bir_lowering=False, debug=False,
                   num_devices=NCORES)
    xp_d = nc.dram_tensor("xp", [CIN, Hp, Hp], DT, kind="ExternalInput")
    wd_d = nc.dram_tensor("wd", [128, CIN * 25], mybir.dt.float32,
                          kind="ExternalInput")
    wen_d = nc.dram_tensor("wen", [128, 25], mybir.dt.float32,
                           kind="ExternalInput")
    out_d = nc.dram_tensor("out", [O_PER, H, W], mybir.dt.float32,
                           kind="ExternalOutput")

    dve_taps = _pick_dve_taps()
    AF = mybir.ActivationFunctionType
    OP = mybir.AluOpType

    def emit_add(is_dve, out_ap, in_ap, w_ap):
        if is_dve:
            nc.vector.tensor_scalar_add(out_ap, in_ap, w_ap)
        else:
            nc.scalar.activation(out_ap, in_ap, AF.Identity, bias=w_ap,
                                 scale=1.0)

    with tile.TileContext(nc) as tc:
        with (
            tc.tile_pool(name="const", bufs=1) as cpool,
            tc.tile_pool(name="x", bufs=1) as xpool,
            tc.tile_pool(name="acc", bufs=1) as apool,
            tc.tile_pool(name="tmp", bufs=4) as tpool,
            tc.tile_pool(name="y", bufs=1) as ypool,
            tc.tile_pool(name="o", bufs=1) as opool,
        ):
            wd_sb = cpool.tile([128, CIN * 25], mybir.dt.float32, tag="wd")
            nc.sync.dma_start(wd_sb[:], wd_d.ap())
            wen_sb = cpool.tile([128, 25], mybir.dt.float32, tag="wen")
            nc.sync.dma_start(wen_sb[:], wen_d.ap())

            for _ in range(reps):
                # ---- load x patches: partition (o,rb) <- padded rows
                # [16rb, 16rb+20) of channel c (same for every o group)
                xpat = []
                for c in range(CIN):
                    xt = xpool.tile([128, 20 * Hp], DT, tag=f"x{c}")
                    src = bass.AP(xp_d, c * Hp * Hp,
                                  [[RPB * Hp, RB], [1, 20 * Hp]])
                    for o in range(O_PER):
                        nc.sync.dma_start(xt[16 * o:16 * o + 16, :], src)
                    xpat.append(xt)

                # ---- dilation ----
                acc = apool.tile([128, RPB * Hp], DT, tag="acc")
                accv = acc[:].rearrange("p (r c) -> p r c", r=RPB, c=Hp)
                acc_int = accv[:, :, PAD:PAD + W]
                acc_c = apool.tile([128, FD], DT, tag="accc")
                accc_v = acc_c[:].rearrange("p (r c) -> p r c", r=RPB, c=W)

                for c in range(CIN):
                    xv = xpat[c][:].rearrange("p (r c2) -> p r c2",
                                              r=20, c2=Hp)
                    dst = acc_int if c == 0 else accc_v
                    for t, (di, dj) in enumerate(TAPS):
                        in0 = xv[:, di:di + RPB, dj:dj + W]
                        wap = wd_sb[:, c * 25 + t:c * 25 + t + 1]
                        is_dve = (c, t) in dve_taps
                        if t == 0:
                            emit_add(is_dve, dst, in0, wap)
                        else:
                            tmp = tpool.tile([128, FD], DT, tag="tmp")
                            tv = tmp[:].rearrange("p (r c2) -> p r c2",
                                                  r=RPB, c2=W)
                            emit_add(is_dve, tv, in0, wap)
                            nc.vector.tensor_tensor(dst, dst, tv, op=OP.max)
                    if c > 0:
                        nc.vector.tensor_tensor(acc_int, acc_int, accc_v,
                                                op=OP.add)

                # replicate edge cols into the pad cols of acc
                for d, s in ((0, PAD), (1, PAD), (Hp - 2, Hp - 3),
                             (Hp - 1, Hp - 3)):
                    nc.scalar.copy(accv[:, :, d:d + 1], accv[:, :, s:s + 1])

                # ---- assemble y patches with row halos (SBUF->SBUF) ----
                ypat = ypool.tile([128, 20 * Hp], DT, tag="ypat")
                # interior rows 2..17
                nc.sync.dma_start(ypat[:, 2 * Hp:18 * Hp], acc[:])
                for o in range(O_PER):
                    p0 = 16 * o
                    # top halo rows 0..1: from (o, rb-1) rows 14..15
                    nc.sync.dma_start(ypat[p0 + 1:p0 + 16, 0:2 * Hp],
                                      acc[p0:p0 + 15, 14 * Hp:16 * Hp])
                    # rb==0: replicate y row 0 (acc row 0)
                    nc.sync.dma_start(ypat[p0:p0 + 1, 0:Hp],
                                      acc[p0:p0 + 1, 0:Hp])
                    nc.sync.dma_start(ypat[p0:p0 + 1, Hp:2 * Hp],
                                      acc[p0:p0 + 1, 0:Hp])
                    # bottom halo rows 18..19: from (o, rb+1) rows 0..1
                    nc.sync.dma_start(ypat[p0:p0 + 15, 18 * Hp:20 * Hp],
                                      acc[p0 + 1:p0 + 16, 0:2 * Hp])
                    # rb==15: replicate y row 255 (acc row 15)
                    nc.sync.dma_start(ypat[p0 + 15:p0 + 16, 18 * Hp:19 * Hp],
                                      acc[p0 + 15:p0 + 16, 15 * Hp:16 * Hp])
                    nc.sync.dma_start(ypat[p0 + 15:p0 + 16, 19 * Hp:20 * Hp],
                                      acc[p0 + 15:p0 + 16, 15 * Hp:16 * Hp])

                # ---- erosion ----
                eacc = apool.tile([128, FD], DT, tag="eacc")
                ev = eacc[:].rearrange("p (r c) -> p r c", r=RPB, c=W)
                yv = ypat[:].rearrange("p (r c) -> p r c", r=20, c=Hp)
                for t, (di, dj) in enumerate(TAPS):
                    in0 = yv[:, di:di + RPB, dj:dj + W]
                    wap = wen_sb[:, t:t + 1]
                    is_dve = (3, t) in dve_taps
                    if t == 0:
                        emit_add(is_dve, ev, in0, wap)
                    else:
                        tmp = tpool.tile([128, FD], DT, tag="tmp")
                        tv = tmp[:].rearrange("p (r c2) -> p r c2",
                                              r=RPB, c2=W)
                        emit_add(is_dve, tv, in0, wap)
                        nc.vector.tensor_tensor(ev, ev, tv, op=OP.min)

                # ---- cast + store ----
                outf = opool.tile([128, FD], mybir.dt.float32, tag="outf")
                nc.vector.tensor_copy(outf[:], eacc[:])
                dst = out_d.ap().rearrange("o (rb r) c -> (o rb) (r c)",
                                           rb=RB)
                nc.sync.dma_start(dst, outf[:])

    nc.compile()
    return nc


_built = {}


def _get_built(reps=1, dt=mybir.dt.float16):
    key = (reps, dt)
    if key not in _built:
        _built[key] = build(reps, dt)
    return _built[key]


def make_in_maps(x, w_d, w_e, np_dt=np.float16):
    x = np.asarray(x, np.float32)
    w_d = np.asarray(w_d, np.float32)
    w_e = np.asarray(w_e, np.float32)
    xp = np.pad(x, ((0, 0), (0, 0), (PAD, PAD), (PAD, PAD)),
                mode="edge").astype(np_dt)
    in_maps = []
    for core in range(NCORES):
        b, oh = divmod(core, 2)
        obase = oh * O_PER
        osel = np.arange(128) // RB + obase          # [128] -> o index
        wd_tab = w_d[osel].reshape(128, CIN * 25).astype(np.float32)
        wen_tab = (-w_e[osel]).reshape(128, 25).astype(np.float32)
        in_maps.append({"xp": xp[b], "wd": wd_tab, "wen": wen_tab})
    return in_maps


def gather(results):
    out = np.empty((B, COUT, H, W), np.float32)
    for core in range(NCORES):
        b, oh = divmod(core, 2)
        out[b, oh * O_PER:(oh + 1) * O_PER] = results[core]["out"]
    return out


def kernel(x, w_d, w_e):
    nc = _get_built()
    in_maps = make_in_maps(x, w_d, w_e)
    res = run_bass_kernel_spmd(nc, in_maps, core_ids=list(range(NCORES)))
    return gather(res.results)
